# revision 22
# baseline (speedup 1.0000x reference)
"""MoE layer (top-2 of 8 experts) on 8 Trainium2 NeuronCores, expert-parallel.

v2 design (vs baseline):
- Replicated routing from a host-transposed fp32 x (no on-chip transposes),
  with the softmax/top-2 vector chain batched over groups of 4 token tiles
  to amortize DVE instruction overheads.
- Routing/compaction/FFN run in ONE pool scope, pipelined by rows of 2048
  tokens: as soon as row r is routed it is compacted and the FFN slot
  columns it completes are published, so FFN chunk c starts after its rows
  instead of after the whole routing prefix.
- Compaction uses 4 rows of 2048 tokens (capacity 576/row -> 2304 slots vs
  2560) with 2 (not 3) local_scatters (token id + bf16 cw).
- FFN matmuls run in bf16 (w1/w3/w2 host-cast; x gathered from a bf16 copy);
  same tensor throughput as fp32r but half the weight-stream DMA (the
  baseline was at the DMA/compute ridge re-streaming fp32 weights per chunk).
- Token-tile transposes on the DMA XBAR (dma_start_transpose), freeing PSUM
  banks so the w2 accumulation runs as contiguous 32-matmul PSUM chains.
- Host scatter-add combine (EP combine), as baseline.

Self-contained: hardcodes shapes for x[4,2048,1024], 8 experts, H=1024,
F=4096, top-2 with renormalized softmax weights.
"""

import os

os.environ.setdefault("JAX_PLATFORMS", "")

import numpy as np
import ml_dtypes

BF16 = ml_dtypes.bfloat16

T, H, F, E = 8192, 1024, 4096, 8
P = 128
NCORES = 8
HC = H // P                  # 8 h-blocks
FT = F // P                  # 32 f-blocks
R = 4                        # compaction rows
RL = T // R                  # 2048 tokens per row
K = 576                      # per-row slot capacity (seed-0 max row load 555)
C = R * K                    # 2304 compact slots per expert
CT = C // P                  # 18 slot tiles
CHUNKS = [512, 512, 512, 512, 256]
assert sum(CHUNKS) == C
NG = 4                       # routing tiles per group
GPR = RL // P // NG          # groups per row (4)

_cache: dict = {}


def _build_nc():
    import concourse.mybir as mybir
    import concourse.tile as tile
    from concourse import bacc
    from concourse.bass import IndirectOffsetOnAxis

    dt = mybir.dt
    Alu = mybir.AluOpType
    Act = mybir.ActivationFunctionType

    nc = bacc.Bacc("TRN2", target_bir_lowering=False, num_devices=NCORES)

    xtt_in = nc.dram_tensor("xtt", [R * GPR, P, NG, HC, P], dt.float32, kind="ExternalInput")
    gwt_in = nc.dram_tensor("gwt", [P, HC, E], dt.float32, kind="ExternalInput")
    esel_in = nc.dram_tensor("esel", [P, E], dt.float32, kind="ExternalInput")
    xg_in = nc.dram_tensor("xg", [T, H], dt.bfloat16, kind="ExternalInput")
    w1_in = nc.dram_tensor("w1t", [FT, P, HC, P], dt.bfloat16, kind="ExternalInput")
    w3_in = nc.dram_tensor("w3t", [FT, P, HC, P], dt.bfloat16, kind="ExternalInput")
    w2_in = nc.dram_tensor("w2t", [2, 8, P, 4, 512], dt.bfloat16, kind="ExternalInput")

    y_out = nc.dram_tensor("y", [C, H], dt.float32, kind="ExternalOutput")
    idx_out = nc.dram_tensor("idx", [C], dt.int32, kind="ExternalOutput")

    # slot column groups completed by each row: row r covers slots
    # [576r, 576(r+1)); column j is ready once slot 128(j+1)-1 is written.
    COLG = [(0, 4), (4, 9), (9, 13), (13, 18)]
    # FFN chunk c needs columns [4c, 4c+nt); gate on the row finishing them.
    CHUNK_ROW = [0, 1, 2, 3, 3]

    with tile.TileContext(nc) as tc:
        with (
            tc.tile_pool(name="const", bufs=1) as cp,
            tc.tile_pool(name="dram", bufs=1, space="DRAM") as dp,
            tc.tile_pool(name="rt", bufs=3) as rm,
            tc.tile_pool(name="cmp", bufs=1) as sm,
            tc.tile_pool(name="f_gx", bufs=3) as fgx,
            tc.tile_pool(name="f_xT", bufs=2) as fxt,
            tc.tile_pool(name="f_hT", bufs=1) as fht,
            tc.tile_pool(name="f_w", bufs=3) as fw,
            tc.tile_pool(name="f_misc", bufs=2) as fm,
            tc.tile_pool(name="ps_rt", bufs=1, space="PSUM") as pr,
            tc.tile_pool(name="ps_f", bufs=1, space="PSUM") as pf,
        ):
            gwt = cp.tile([P, HC, E], dt.float32)
            nc.sync.dma_start(gwt[:], gwt_in[:])
            esel = cp.tile([P, E], dt.float32)
            nc.sync.dma_start(esel[:], esel_in[:])

            cwtok = dp.tile([T], dt.float32)
            cw128 = cp.tile([P, CT], dt.float32)
            idx_i = cp.tile([P, CT], dt.int32)
            idg_i = cp.tile([P, CT], dt.int32)

            cw_all = rm.tile([P, T // P], dt.float32, tag="cwall", bufs=1)
            zeros = sm.tile([16, RL], dt.float32)
            nc.vector.memset(zeros[:], 0.0)
            # iota on partition 0: value j+1 (row token offset added later)
            iop1 = sm.tile([16, RL], dt.int32, tag="scn")  # reuse scn slot
            nc.gpsimd.iota(iop1[:], pattern=[[1, RL]], base=1, channel_multiplier=0)
            idsp1 = sm.tile([16, RL], dt.uint16)
            nc.vector.tensor_copy(idsp1[:], iop1[:])
            idxflat = dp.tile([C], dt.float32)
            idgflat = dp.tile([C], dt.float32)
            cwflat = dp.tile([C], dt.float32)
            ids128 = sm.tile([P, CT], dt.float32)
            idg128 = sm.tile([P, CT], dt.float32)

            # =========== routing + compaction for one row of tokens ==========
            def route_and_compact_row(r):
                for gg in range(GPR):
                    g = r * GPR + gg
                    xtg = rm.tile([P, NG, HC, P], dt.float32, tag="xtt", bufs=2)
                    # split each group's 2.1MB across both HWDGE queues
                    nc.sync.dma_start(xtg[:, : NG // 2], xtt_in[g, :, : NG // 2])
                    nc.scalar.dma_start(xtg[:, NG // 2 :], xtt_in[g, :, NG // 2 :])
                    gp0 = pr.tile([P, NG, E], dt.float32, tag="gp0", bufs=1)
                    gp1 = pr.tile([P, NG, E], dt.float32, tag="gp1", bufs=1)
                    for t in range(NG):
                        # gate logits in 2 split-K partials (precision: top-2/3
                        # logit gaps go down to ~3e-6; must match the fp32 ref)
                        for k, gp in ((0, gp0), (1, gp1)):
                            for s in range(4):
                                nc.tensor.matmul(
                                    gp[:, t, :], xtg[:, t, 4 * k + s, :],
                                    gwt[:, 4 * k + s, :],
                                    start=(s == 0), stop=(s == 3),
                                )
                    lg = rm.tile([P, NG, E], dt.float32, tag="lg")
                    nc.vector.tensor_copy(lg[:], gp0[:])
                    nc.vector.tensor_tensor(lg[:], lg[:], gp1[:], op=Alu.add)

                    mx = rm.tile([P, NG, 8], dt.float32, tag="mx")
                    for t in range(NG):
                        nc.vector.max(mx[:, t, :], lg[:, t, :])
                    # sig = sigmoid(2*(lg - (mx0+mx1)/2))
                    negs = rm.tile([P, NG, 1], dt.float32, tag="negs")
                    nc.vector.tensor_tensor(
                        negs[:], mx[:, :, 0:1], mx[:, :, 1:2], op=Alu.add
                    )
                    nc.vector.tensor_scalar_mul(negs[:], negs[:], -0.5)
                    arg = rm.tile([P, NG, E], dt.float32, tag="arg")
                    nc.vector.tensor_tensor(
                        arg[:], lg[:], negs[:].broadcast_to([P, NG, E]), op=Alu.add
                    )
                    sig = rm.tile([P, NG, E], dt.float32, tag="sig")
                    nc.scalar.activation(sig[:], arg[:], Act.Sigmoid, scale=2.0)
                    msk = rm.tile([P, NG, E], dt.float32, tag="msk")
                    nc.vector.tensor_tensor(
                        msk[:], lg[:], mx[:, :, 1:2].broadcast_to([P, NG, E]),
                        op=Alu.is_ge,
                    )
                    cw8 = rm.tile([P, NG, E], dt.float32, tag="cw8")
                    nc.vector.tensor_tensor(cw8[:], sig[:], msk[:], op=Alu.mult)
                    nc.vector.tensor_tensor(
                        cw8[:], cw8[:],
                        esel[:].rearrange("p (o e) -> p o e", o=1)
                        .broadcast_to([P, NG, E]),
                        op=Alu.mult,
                    )
                    nc.vector.tensor_reduce(
                        cw_all[:, g * NG : (g + 1) * NG]
                        .rearrange("p (t o) -> p t o", o=1),
                        cw8[:], axis=mybir.AxisListType.X, op=Alu.add,
                    )

                # ---- compact row r (tokens [2048r, 2048(r+1))) ----
                # per-row tiles live on partitions 0-15 with the real data on
                # partition 0; garbage partitions self-filter (their mask is
                # 0/1, the scan gives unique in-range-or-ignored positions,
                # and only partition 0 of the compact tiles is ever read).
                nc.sync.dma_start(
                    cwtok[RL * r : RL * (r + 1)].rearrange("(i p) -> p i", p=P),
                    cw_all[:, 16 * r : 16 * (r + 1)],
                )
                cwr = sm.tile([16, RL], dt.float32, tag="cwr")
                nc.scalar.dma_start(
                    cwr[:].rearrange("(o s) f -> o s f", o=1)[:, 0, :],
                    cwtok[RL * r : RL * (r + 1)].rearrange("(o f) -> o f", o=1),
                )
                cwb = sm.tile([16, RL], dt.bfloat16, tag="cwb")
                nc.vector.tensor_copy(cwb[:], cwr[:])
                mask = sm.tile([16, RL], dt.float32, tag="mask")
                nc.vector.tensor_scalar(mask[:], cwr[:], 0.0, None, op0=Alu.is_gt)
                scn = sm.tile([16, RL], dt.float32, tag="scn")
                nc.vector.tensor_tensor_scan(
                    scn[:], mask[:], zeros[:], 0.0, Alu.add, Alu.add
                )
                # pos = scn - mask; posf = (pos+1)*mask*inb - 1  (in-place in scn)
                inb = sm.tile([16, RL], dt.float32, tag="cwr")  # reuse cwr slot
                nc.vector.tensor_tensor(scn[:], scn[:], mask[:], op=Alu.subtract)
                nc.vector.tensor_scalar(inb[:], scn[:], float(K - 1), None, op0=Alu.is_le)
                nc.vector.tensor_scalar(scn[:], scn[:], 1.0, None, op0=Alu.add)
                nc.vector.tensor_tensor(scn[:], scn[:], mask[:], op=Alu.mult)
                nc.vector.tensor_tensor(scn[:], scn[:], inb[:], op=Alu.mult)
                nc.vector.tensor_scalar(scn[:], scn[:], 1.0, None, op0=Alu.subtract)
                posi = sm.tile([16, RL], dt.int16, tag="posi")
                nc.vector.tensor_copy(posi[:], scn[:])

                pc_id = sm.tile([16, K], dt.uint16, tag="pcid")
                pc_cw = sm.tile([16, K], dt.uint16, tag="pccw")
                nc.gpsimd.local_scatter(pc_id[:], idsp1[:], posi[:], 16, K, RL)
                nc.gpsimd.local_scatter(
                    pc_cw[:], cwb[:].bitcast(dt.uint16), posi[:], 16, K, RL
                )

                # real slot: id j+1 -> 2048r + j; empty slot (0) -> 8192
                idf = sm.tile([16, K], dt.float32, tag="idf")
                nc.vector.tensor_copy(idf[:], pc_id[:])
                zt = sm.tile([16, K], dt.float32, tag="zt")
                nc.vector.tensor_scalar(
                    zt[:], idf[:], 0.0, float(8193 - RL * r),
                    op0=Alu.is_equal, op1=Alu.mult,
                )
                nc.vector.tensor_tensor(idf[:], idf[:], zt[:], op=Alu.add)
                nc.vector.tensor_scalar(
                    idf[:], idf[:], float(RL * r - 1), None, op0=Alu.add
                )
                idgf = sm.tile([16, K], dt.float32, tag="idgf")
                nc.vector.tensor_scalar_min(idgf[:], idf[:], float(T - 1))
                cwf = sm.tile([16, K], dt.float32, tag="cwf")
                nc.vector.tensor_copy(cwf[:], pc_cw[:].bitcast(dt.bfloat16))

                def _row(tile_ap):
                    return tile_ap.rearrange("(o s) f -> o s f", o=1)[:, 0, :]

                fsl = slice(K * r, K * (r + 1))
                nc.sync.dma_start(
                    idxflat[fsl].rearrange("(o f) -> o f", o=1), _row(idf[:])
                )
                nc.sync.dma_start(
                    idgflat[fsl].rearrange("(o f) -> o f", o=1), _row(idgf[:])
                )
                nc.sync.dma_start(
                    cwflat[fsl].rearrange("(o f) -> o f", o=1), _row(cwf[:])
                )

                # ---- publish the slot columns completed by this row ----
                a, b = COLG[r]
                csl = slice(a, b)
                nc.sync.dma_start(
                    ids128[:, csl], idxflat[:].rearrange("(j p) -> p j", p=P)[:, csl]
                )
                nc.sync.dma_start(
                    idg128[:, csl], idgflat[:].rearrange("(j p) -> p j", p=P)[:, csl]
                )
                nc.scalar.dma_start(
                    cw128[:, csl], cwflat[:].rearrange("(j p) -> p j", p=P)[:, csl]
                )
                nc.vector.tensor_copy(idx_i[:, csl], ids128[:, csl])
                nc.sync.dma_start(
                    idx_out[:].rearrange("(j p) -> p j", p=P)[:, csl], idx_i[:, csl]
                )
                nc.vector.tensor_copy(idg_i[:, csl], idg128[:, csl])

            # ================= FFN on one chunk of compact slots =============
            def ffn_chunk(jt0, tc_size):
                nt = tc_size // P
                xT = fxt.tile([P, HC, 512], dt.bfloat16, tag="xT")
                for jj in range(nt):
                    gx = fgx.tile([P, H], dt.bfloat16, tag="gx")
                    nc.gpsimd.indirect_dma_start(
                        out=gx[:],
                        out_offset=None,
                        in_=xg_in[:],
                        in_offset=IndirectOffsetOnAxis(
                            ap=idg_i[:, jt0 + jj : jt0 + jj + 1], axis=0
                        ),
                    )
                    nc.scalar.dma_start_transpose(
                        xT[:, :, jj * P : (jj + 1) * P], gx[:]
                    )

                hT = fht.tile([P, FT, 512], dt.bfloat16, tag="hT", bufs=1)
                for ft in range(FT):
                    w1t = fw.tile([P, HC, P], dt.bfloat16, tag="w1")
                    nc.sync.dma_start(w1t[:], w1_in[ft])
                    w3t = fw.tile([P, HC, P], dt.bfloat16, tag="w3")
                    nc.scalar.dma_start(w3t[:], w3_in[ft])
                    pa = pf.tile([P, 512], dt.float32, tag="pa", bufs=2)
                    pb = pf.tile([P, 512], dt.float32, tag="pb", bufs=2)
                    for hc in range(HC):
                        nc.tensor.matmul(
                            pa[:, :tc_size], w1t[:, hc, :], xT[:, hc, :tc_size],
                            start=(hc == 0), stop=(hc == HC - 1),
                        )
                    for hc in range(HC):
                        nc.tensor.matmul(
                            pb[:, :tc_size], w3t[:, hc, :], xT[:, hc, :tc_size],
                            start=(hc == 0), stop=(hc == HC - 1),
                        )
                    sl = fm.tile([P, 512], dt.float32, tag="sl")
                    nc.scalar.activation(sl[:, :tc_size], pa[:, :tc_size], Act.Silu)
                    nc.vector.tensor_tensor(
                        hT[:, ft, :tc_size], sl[:, :tc_size], pb[:, :tc_size],
                        op=Alu.mult,
                    )

                for hn in range(2):
                    w2_tiles = []
                    for ftg in range(8):
                        w2t = fw.tile([P, 4, 512], dt.bfloat16, tag="w2", bufs=10)
                        (nc.sync if ftg % 2 else nc.scalar).dma_start(
                            w2t[:], w2_in[hn, ftg]
                        )
                        w2_tiles.append(w2t)
                    for ts in range(nt):
                        py = pf.tile([P, 512], dt.float32, tag="py", bufs=2)
                        for ftg in range(8):
                            for j4 in range(4):
                                nc.tensor.matmul(
                                    py[:],
                                    hT[:, ftg * 4 + j4, ts * P : (ts + 1) * P],
                                    w2_tiles[ftg][:, j4, :],
                                    start=(ftg == 0 and j4 == 0),
                                    stop=(ftg == 7 and j4 == 3),
                                )
                        ysb = fm.tile([P, 512], dt.float32, tag="ysb")
                        nc.vector.tensor_scalar(
                            ysb[:], py[:],
                            cw128[:, jt0 + ts : jt0 + ts + 1], None,
                            op0=Alu.mult,
                        )
                        nc.sync.dma_start(
                            y_out[:].rearrange("(a p) h -> p a h", p=P)[
                                :, jt0 + ts, hn * 512 : (hn + 1) * 512
                            ],
                            ysb[:],
                        )

            # ======================= pipelined schedule ======================
            # rows 0..3 routed+compacted in order; FFN chunk c is emitted
            # right after the row it needs, so the tile scheduler can overlap
            # chunk c with the routing of rows > CHUNK_ROW[c].
            jt0s = np.cumsum([0] + CHUNKS[:-1]).tolist()
            next_chunk = 0
            for r in range(R):
                route_and_compact_row(r)
                while next_chunk < len(CHUNKS) and CHUNK_ROW[next_chunk] == r:
                    ffn_chunk(jt0s[next_chunk] // P, CHUNKS[next_chunk])
                    next_chunk += 1

    nc.finalize()
    return nc


def _prep_shared(xf, gate_w, w1, w2, w3):
    """Inputs independent of the core id (cast/transpose once)."""
    gwt = np.ascontiguousarray(
        gate_w.T.reshape(HC, P, E).transpose(1, 0, 2)
    ).astype(np.float32)
    xg = xf.astype(BF16)
    xtt = np.ascontiguousarray(
        xf.reshape(R * GPR, NG, P, HC, P).transpose(0, 4, 1, 3, 2)
    ).astype(np.float32)
    w1t, w3t, w2t = [], [], []
    for e in range(NCORES):
        w1t.append(np.ascontiguousarray(
            w1[e].reshape(HC, P, FT, P).transpose(2, 1, 0, 3)).astype(BF16))
        w3t.append(np.ascontiguousarray(
            w3[e].reshape(HC, P, FT, P).transpose(2, 1, 0, 3)).astype(BF16))
        w2t.append(np.ascontiguousarray(
            w2[e].reshape(8, 4, P, 2, 512).transpose(3, 0, 2, 1, 4)).astype(BF16))
    return gwt, xg, xtt, w1t, w3t, w2t


def _prep_core_inputs(shared, xf, e):
    gwt, xg, xtt, w1t, w3t, w2t = shared
    esel = np.zeros((P, E), dtype=np.float32)
    esel[:, e] = 1.0
    return {
        "xtt": xtt, "gwt": gwt, "esel": esel, "xg": xg,
        "w1t": w1t[e], "w3t": w3t[e], "w2t": w2t[e],
    }


def _run(inputs, trace=False):
    from concourse.bass_utils import run_bass_kernel_spmd

    x = np.ascontiguousarray(np.asarray(inputs["x"], dtype=np.float32))
    gate_w = np.ascontiguousarray(np.asarray(inputs["gate_w"], dtype=np.float32))
    w1 = np.ascontiguousarray(np.asarray(inputs["w1"], dtype=np.float32))
    w2 = np.ascontiguousarray(np.asarray(inputs["w2"], dtype=np.float32))
    w3 = np.ascontiguousarray(np.asarray(inputs["w3"], dtype=np.float32))
    xf = x.reshape(T, H)

    # capacity safety check (host-side routing estimate; K has margin over
    # the boundary-rounding uncertainty of this estimate)
    logits = xf @ gate_w.T
    m2 = np.sort(logits, axis=1)[:, -2:-1]
    mask = logits >= m2
    pp = mask.reshape(R, RL, E).sum(axis=1)
    if pp.max() > K:
        raise RuntimeError(
            f"per-row expert token count {pp.max()} exceeds compiled "
            f"capacity K={K}; rebuild kernel.py with a larger K"
        )

    if "nc" not in _cache:
        _cache["nc"] = _build_nc()
    nc = _cache["nc"]

    shared = _prep_shared(xf, gate_w, w1, w2, w3)
    in_maps = [_prep_core_inputs(shared, xf, e) for e in range(NCORES)]
    res = run_bass_kernel_spmd(nc, in_maps, core_ids=list(range(NCORES)), trace=trace)

    out = np.zeros((T + 1, H), dtype=np.float32)
    for e in range(NCORES):
        idx = res.results[e]["idx"]
        y = res.results[e]["y"]
        out[idx] += y
    return out[:T].reshape(x.shape), res


def kernel(**inputs) -> np.ndarray:
    out, _ = _run(inputs, trace=False)
    return out


# revision 23
# speedup vs baseline: 1.1376x; 1.1376x over previous
"""MoE layer (top-2 of 8 experts) on 8 Trainium2 NeuronCores, expert-parallel.

v2 design (vs baseline):
- Replicated routing from a host-transposed fp32 x (no on-chip transposes),
  with the softmax/top-2 vector chain batched over groups of 4 token tiles
  to amortize DVE instruction overheads.
- Routing/compaction/FFN run in ONE pool scope, pipelined by rows of 2048
  tokens: as soon as row r is routed it is compacted and the FFN slot
  columns it completes are published, so FFN chunk c starts after its rows
  instead of after the whole routing prefix.
- Compaction uses 4 rows of 2048 tokens (capacity 576/row -> 2304 slots vs
  2560) with 2 (not 3) local_scatters (token id + bf16 cw).
- FFN matmuls run in bf16 (w1/w3/w2 host-cast; x gathered from a bf16 copy);
  same tensor throughput as fp32r but half the weight-stream DMA (the
  baseline was at the DMA/compute ridge re-streaming fp32 weights per chunk).
- Token-tile transposes on the DMA XBAR (dma_start_transpose), freeing PSUM
  banks so the w2 accumulation runs as contiguous 32-matmul PSUM chains.
- Host scatter-add combine (EP combine), as baseline.

Self-contained: hardcodes shapes for x[4,2048,1024], 8 experts, H=1024,
F=4096, top-2 with renormalized softmax weights.
"""

import os

os.environ.setdefault("JAX_PLATFORMS", "")

import numpy as np
import ml_dtypes

BF16 = ml_dtypes.bfloat16

T, H, F, E = 8192, 1024, 4096, 8
P = 128
NCORES = 8
HC = H // P                  # 8 h-blocks
FT = F // P                  # 32 f-blocks
R = 4                        # compaction rows
RL = T // R                  # 2048 tokens per row
K = 576                      # per-row slot capacity (seed-0 max row load 555)
C = R * K                    # 2304 compact slots per expert
CT = C // P                  # 18 slot tiles
CHUNKS = [512, 512, 512, 512, 256]
assert sum(CHUNKS) == C
NG = 4                       # routing tiles per group
GPR = RL // P // NG          # groups per row (4)

_cache: dict = {}


def _build_nc():
    import concourse.mybir as mybir
    import concourse.tile as tile
    from concourse import bacc
    from concourse.bass import IndirectOffsetOnAxis

    dt = mybir.dt
    Alu = mybir.AluOpType
    Act = mybir.ActivationFunctionType

    nc = bacc.Bacc("TRN2", target_bir_lowering=False, num_devices=NCORES)

    xtt_in = nc.dram_tensor("xtt", [R * GPR, P, NG, HC, P], dt.float32, kind="ExternalInput")
    gwt_in = nc.dram_tensor("gwt", [P, HC, E], dt.float32, kind="ExternalInput")
    esel_in = nc.dram_tensor("esel", [P, E], dt.float32, kind="ExternalInput")
    xg_in = nc.dram_tensor("xg", [T, H], dt.bfloat16, kind="ExternalInput")
    w1_in = nc.dram_tensor("w1t", [FT, P, HC, P], dt.bfloat16, kind="ExternalInput")
    w3_in = nc.dram_tensor("w3t", [FT, P, HC, P], dt.bfloat16, kind="ExternalInput")
    w2_in = nc.dram_tensor("w2t", [2, 8, P, 4, 512], dt.bfloat16, kind="ExternalInput")

    y_out = nc.dram_tensor("y", [C, H], dt.float32, kind="ExternalOutput")
    idx_out = nc.dram_tensor("idx", [C], dt.int32, kind="ExternalOutput")

    # slot column groups completed by each row: row r covers slots
    # [576r, 576(r+1)); column j is ready once slot 128(j+1)-1 is written.
    COLG = [(0, 4), (4, 9), (9, 13), (13, 18)]
    # FFN chunk c needs columns [4c, 4c+nt); gate on the row finishing them.
    CHUNK_ROW = [0, 1, 2, 3, 3]

    with tile.TileContext(nc) as tc:
        with (
            tc.tile_pool(name="const", bufs=1) as cp,
            tc.tile_pool(name="dram", bufs=1, space="DRAM") as dp,
            tc.tile_pool(name="rt", bufs=3) as rm,
            tc.tile_pool(name="cmp", bufs=1) as sm,
            tc.tile_pool(name="f_gx", bufs=3) as fgx,
            tc.tile_pool(name="f_xT", bufs=2) as fxt,
            tc.tile_pool(name="f_hT", bufs=1) as fht,
            tc.tile_pool(name="f_w", bufs=3) as fw,
            tc.tile_pool(name="f_misc", bufs=2) as fm,
            tc.tile_pool(name="ps_rt", bufs=1, space="PSUM") as pr,
            tc.tile_pool(name="ps_f", bufs=1, space="PSUM") as pf,
        ):
            gwt = cp.tile([P, HC, E], dt.float32)
            nc.sync.dma_start(gwt[:], gwt_in[:])
            esel = cp.tile([P, E], dt.float32)
            nc.sync.dma_start(esel[:], esel_in[:])

            cwtok = dp.tile([T], dt.float32)
            cw128 = cp.tile([P, CT], dt.float32)
            idx_i = cp.tile([P, CT], dt.int32)
            idg_i = cp.tile([P, CT], dt.int32)

            cw_all = rm.tile([P, T // P], dt.float32, tag="cwall", bufs=1)
            zeros = sm.tile([16, RL], dt.float32)
            nc.vector.memset(zeros[:], 0.0)
            # iota on partition 0: value j+1 (row token offset added later)
            iop1 = sm.tile([16, RL], dt.int32, tag="scn")  # reuse scn slot
            nc.gpsimd.iota(iop1[:], pattern=[[1, RL]], base=1, channel_multiplier=0)
            idsp1 = sm.tile([16, RL], dt.uint16)
            nc.vector.tensor_copy(idsp1[:], iop1[:])
            idxflat = dp.tile([C], dt.float32)
            idgflat = dp.tile([C], dt.float32)
            cwflat = dp.tile([C], dt.float32)
            ids128 = sm.tile([P, CT], dt.float32)
            idg128 = sm.tile([P, CT], dt.float32)

            # =========== routing + compaction for one row of tokens ==========
            def route_and_compact_row(r):
                for gg in range(GPR):
                    g = r * GPR + gg
                    xtg = rm.tile([P, NG, HC, P], dt.float32, tag="xtt", bufs=2)
                    if r == 0:
                        # critical prefix: split across both HWDGE queues
                        nc.sync.dma_start(xtg[:, : NG // 2], xtt_in[g, :, : NG // 2])
                        nc.scalar.dma_start(xtg[:, NG // 2 :], xtt_in[g, :, NG // 2 :])
                    else:
                        # later rows stream on the gpsimd SWDGE queue so they
                        # are not stuck behind the FFN weight DMAs
                        nc.gpsimd.dma_start(xtg[:], xtt_in[g])
                    gp0 = pr.tile([P, NG, E], dt.float32, tag="gp0", bufs=1)
                    gp1 = pr.tile([P, NG, E], dt.float32, tag="gp1", bufs=1)
                    for t in range(NG):
                        # gate logits in 2 split-K partials (precision: top-2/3
                        # logit gaps go down to ~3e-6; must match the fp32 ref)
                        for k, gp in ((0, gp0), (1, gp1)):
                            for s in range(4):
                                nc.tensor.matmul(
                                    gp[:, t, :], xtg[:, t, 4 * k + s, :],
                                    gwt[:, 4 * k + s, :],
                                    start=(s == 0), stop=(s == 3),
                                )
                    lg = rm.tile([P, NG, E], dt.float32, tag="lg")
                    nc.vector.tensor_copy(lg[:], gp0[:])
                    nc.vector.tensor_tensor(lg[:], lg[:], gp1[:], op=Alu.add)

                    mx = rm.tile([P, NG, 8], dt.float32, tag="mx")
                    for t in range(NG):
                        nc.vector.max(mx[:, t, :], lg[:, t, :])
                    # sig = sigmoid(2*(lg - (mx0+mx1)/2))
                    negs = rm.tile([P, NG, 1], dt.float32, tag="negs")
                    nc.vector.tensor_tensor(
                        negs[:], mx[:, :, 0:1], mx[:, :, 1:2], op=Alu.add
                    )
                    nc.vector.tensor_scalar_mul(negs[:], negs[:], -0.5)
                    arg = rm.tile([P, NG, E], dt.float32, tag="arg")
                    nc.vector.tensor_tensor(
                        arg[:], lg[:], negs[:].broadcast_to([P, NG, E]), op=Alu.add
                    )
                    sig = rm.tile([P, NG, E], dt.float32, tag="sig")
                    nc.scalar.activation(sig[:], arg[:], Act.Sigmoid, scale=2.0)
                    msk = rm.tile([P, NG, E], dt.float32, tag="msk")
                    nc.vector.tensor_tensor(
                        msk[:], lg[:], mx[:, :, 1:2].broadcast_to([P, NG, E]),
                        op=Alu.is_ge,
                    )
                    cw8 = rm.tile([P, NG, E], dt.float32, tag="cw8")
                    nc.vector.tensor_tensor(cw8[:], sig[:], msk[:], op=Alu.mult)
                    nc.vector.tensor_tensor(
                        cw8[:], cw8[:],
                        esel[:].rearrange("p (o e) -> p o e", o=1)
                        .broadcast_to([P, NG, E]),
                        op=Alu.mult,
                    )
                    nc.vector.tensor_reduce(
                        cw_all[:, g * NG : (g + 1) * NG]
                        .rearrange("p (t o) -> p t o", o=1),
                        cw8[:], axis=mybir.AxisListType.X, op=Alu.add,
                    )

                # ---- compact row r (tokens [2048r, 2048(r+1))) ----
                # per-row tiles live on partitions 0-15 with the real data on
                # partition 0; garbage partitions self-filter (their mask is
                # 0/1, the scan gives unique in-range-or-ignored positions,
                # and only partition 0 of the compact tiles is ever read).
                nc.sync.dma_start(
                    cwtok[RL * r : RL * (r + 1)].rearrange("(i p) -> p i", p=P),
                    cw_all[:, 16 * r : 16 * (r + 1)],
                )
                cwr = sm.tile([16, RL], dt.float32, tag="cwr")
                nc.scalar.dma_start(
                    cwr[:].rearrange("(o s) f -> o s f", o=1)[:, 0, :],
                    cwtok[RL * r : RL * (r + 1)].rearrange("(o f) -> o f", o=1),
                )
                cwb = sm.tile([16, RL], dt.bfloat16, tag="cwb")
                nc.vector.tensor_copy(cwb[:], cwr[:])
                mask = sm.tile([16, RL], dt.float32, tag="mask")
                nc.vector.tensor_scalar(mask[:], cwr[:], 0.0, None, op0=Alu.is_gt)
                scn = sm.tile([16, RL], dt.float32, tag="scn")
                nc.vector.tensor_tensor_scan(
                    scn[:], mask[:], zeros[:], 0.0, Alu.add, Alu.add
                )
                # pos = scn - mask; posf = (pos+1)*mask*inb - 1  (in-place in scn)
                inb = sm.tile([16, RL], dt.float32, tag="cwr")  # reuse cwr slot
                nc.vector.tensor_tensor(scn[:], scn[:], mask[:], op=Alu.subtract)
                nc.vector.tensor_scalar(inb[:], scn[:], float(K - 1), None, op0=Alu.is_le)
                nc.vector.tensor_scalar(scn[:], scn[:], 1.0, None, op0=Alu.add)
                nc.vector.tensor_tensor(scn[:], scn[:], mask[:], op=Alu.mult)
                nc.vector.tensor_tensor(scn[:], scn[:], inb[:], op=Alu.mult)
                nc.vector.tensor_scalar(scn[:], scn[:], 1.0, None, op0=Alu.subtract)
                posi = sm.tile([16, RL], dt.int16, tag="posi")
                nc.vector.tensor_copy(posi[:], scn[:])

                pc_id = sm.tile([16, K], dt.uint16, tag="pcid")
                pc_cw = sm.tile([16, K], dt.uint16, tag="pccw")
                nc.gpsimd.local_scatter(pc_id[:], idsp1[:], posi[:], 16, K, RL)
                nc.gpsimd.local_scatter(
                    pc_cw[:], cwb[:].bitcast(dt.uint16), posi[:], 16, K, RL
                )

                # real slot: id j+1 -> 2048r + j; empty slot (0) -> 8192
                idf = sm.tile([16, K], dt.float32, tag="idf")
                nc.vector.tensor_copy(idf[:], pc_id[:])
                zt = sm.tile([16, K], dt.float32, tag="zt")
                nc.vector.tensor_scalar(
                    zt[:], idf[:], 0.0, float(8193 - RL * r),
                    op0=Alu.is_equal, op1=Alu.mult,
                )
                nc.vector.tensor_tensor(idf[:], idf[:], zt[:], op=Alu.add)
                nc.vector.tensor_scalar(
                    idf[:], idf[:], float(RL * r - 1), None, op0=Alu.add
                )
                idgf = sm.tile([16, K], dt.float32, tag="idgf")
                nc.vector.tensor_scalar_min(idgf[:], idf[:], float(T - 1))
                cwf = sm.tile([16, K], dt.float32, tag="cwf")
                nc.vector.tensor_copy(cwf[:], pc_cw[:].bitcast(dt.bfloat16))

                def _row(tile_ap):
                    return tile_ap.rearrange("(o s) f -> o s f", o=1)[:, 0, :]

                fsl = slice(K * r, K * (r + 1))
                nc.sync.dma_start(
                    idxflat[fsl].rearrange("(o f) -> o f", o=1), _row(idf[:])
                )
                nc.sync.dma_start(
                    idgflat[fsl].rearrange("(o f) -> o f", o=1), _row(idgf[:])
                )
                nc.sync.dma_start(
                    cwflat[fsl].rearrange("(o f) -> o f", o=1), _row(cwf[:])
                )

                # ---- publish the slot columns completed by this row ----
                a, b = COLG[r]
                csl = slice(a, b)
                nc.sync.dma_start(
                    ids128[:, csl], idxflat[:].rearrange("(j p) -> p j", p=P)[:, csl]
                )
                nc.sync.dma_start(
                    idg128[:, csl], idgflat[:].rearrange("(j p) -> p j", p=P)[:, csl]
                )
                nc.scalar.dma_start(
                    cw128[:, csl], cwflat[:].rearrange("(j p) -> p j", p=P)[:, csl]
                )
                nc.vector.tensor_copy(idx_i[:, csl], ids128[:, csl])
                nc.sync.dma_start(
                    idx_out[:].rearrange("(j p) -> p j", p=P)[:, csl], idx_i[:, csl]
                )
                nc.vector.tensor_copy(idg_i[:, csl], idg128[:, csl])

            # ================= FFN on one chunk of compact slots =============
            def ffn_chunk(jt0, tc_size):
                nt = tc_size // P
                xT = fxt.tile([P, HC, 512], dt.bfloat16, tag="xT")
                for jj in range(nt):
                    gx = fgx.tile([P, H], dt.bfloat16, tag="gx")
                    nc.gpsimd.indirect_dma_start(
                        out=gx[:],
                        out_offset=None,
                        in_=xg_in[:],
                        in_offset=IndirectOffsetOnAxis(
                            ap=idg_i[:, jt0 + jj : jt0 + jj + 1], axis=0
                        ),
                    )
                    nc.scalar.dma_start_transpose(
                        xT[:, :, jj * P : (jj + 1) * P], gx[:]
                    )

                hT = fht.tile([P, FT, 512], dt.bfloat16, tag="hT", bufs=1)
                for ft in range(FT):
                    w1t = fw.tile([P, HC, P], dt.bfloat16, tag="w1")
                    nc.sync.dma_start(w1t[:], w1_in[ft])
                    w3t = fw.tile([P, HC, P], dt.bfloat16, tag="w3")
                    nc.scalar.dma_start(w3t[:], w3_in[ft])
                    pa = pf.tile([P, 512], dt.float32, tag="pa", bufs=2)
                    pb = pf.tile([P, 512], dt.float32, tag="pb", bufs=2)
                    for hc in range(HC):
                        nc.tensor.matmul(
                            pa[:, :tc_size], w1t[:, hc, :], xT[:, hc, :tc_size],
                            start=(hc == 0), stop=(hc == HC - 1),
                        )
                    for hc in range(HC):
                        nc.tensor.matmul(
                            pb[:, :tc_size], w3t[:, hc, :], xT[:, hc, :tc_size],
                            start=(hc == 0), stop=(hc == HC - 1),
                        )
                    sl = fm.tile([P, 512], dt.float32, tag="sl")
                    nc.scalar.activation(sl[:, :tc_size], pa[:, :tc_size], Act.Silu)
                    nc.vector.tensor_tensor(
                        hT[:, ft, :tc_size], sl[:, :tc_size], pb[:, :tc_size],
                        op=Alu.mult,
                    )

                for hn in range(2):
                    w2_tiles = []
                    for ftg in range(8):
                        w2t = fw.tile([P, 4, 512], dt.bfloat16, tag="w2", bufs=10)
                        (nc.sync if ftg % 2 else nc.scalar).dma_start(
                            w2t[:], w2_in[hn, ftg]
                        )
                        w2_tiles.append(w2t)
                    for ts in range(nt):
                        py = pf.tile([P, 512], dt.float32, tag="py", bufs=2)
                        for ftg in range(8):
                            for j4 in range(4):
                                nc.tensor.matmul(
                                    py[:],
                                    hT[:, ftg * 4 + j4, ts * P : (ts + 1) * P],
                                    w2_tiles[ftg][:, j4, :],
                                    start=(ftg == 0 and j4 == 0),
                                    stop=(ftg == 7 and j4 == 3),
                                )
                        ysb = fm.tile([P, 512], dt.float32, tag="ysb")
                        nc.vector.tensor_scalar(
                            ysb[:], py[:],
                            cw128[:, jt0 + ts : jt0 + ts + 1], None,
                            op0=Alu.mult,
                        )
                        nc.sync.dma_start(
                            y_out[:].rearrange("(a p) h -> p a h", p=P)[
                                :, jt0 + ts, hn * 512 : (hn + 1) * 512
                            ],
                            ysb[:],
                        )

            # ======================= pipelined schedule ======================
            # rows 0..3 routed+compacted in order; FFN chunk c is emitted
            # right after the row it needs, so the tile scheduler can overlap
            # chunk c with the routing of rows > CHUNK_ROW[c].
            jt0s = np.cumsum([0] + CHUNKS[:-1]).tolist()
            next_chunk = 0
            for r in range(R):
                route_and_compact_row(r)
                while next_chunk < len(CHUNKS) and CHUNK_ROW[next_chunk] == r:
                    ffn_chunk(jt0s[next_chunk] // P, CHUNKS[next_chunk])
                    next_chunk += 1

    nc.finalize()
    return nc


def _prep_shared(xf, gate_w, w1, w2, w3):
    """Inputs independent of the core id (cast/transpose once)."""
    gwt = np.ascontiguousarray(
        gate_w.T.reshape(HC, P, E).transpose(1, 0, 2)
    ).astype(np.float32)
    xg = xf.astype(BF16)
    xtt = np.ascontiguousarray(
        xf.reshape(R * GPR, NG, P, HC, P).transpose(0, 4, 1, 3, 2)
    ).astype(np.float32)
    w1t, w3t, w2t = [], [], []
    for e in range(NCORES):
        w1t.append(np.ascontiguousarray(
            w1[e].reshape(HC, P, FT, P).transpose(2, 1, 0, 3)).astype(BF16))
        w3t.append(np.ascontiguousarray(
            w3[e].reshape(HC, P, FT, P).transpose(2, 1, 0, 3)).astype(BF16))
        w2t.append(np.ascontiguousarray(
            w2[e].reshape(8, 4, P, 2, 512).transpose(3, 0, 2, 1, 4)).astype(BF16))
    return gwt, xg, xtt, w1t, w3t, w2t


def _prep_core_inputs(shared, xf, e):
    gwt, xg, xtt, w1t, w3t, w2t = shared
    esel = np.zeros((P, E), dtype=np.float32)
    esel[:, e] = 1.0
    return {
        "xtt": xtt, "gwt": gwt, "esel": esel, "xg": xg,
        "w1t": w1t[e], "w3t": w3t[e], "w2t": w2t[e],
    }


def _run(inputs, trace=False):
    from concourse.bass_utils import run_bass_kernel_spmd

    x = np.ascontiguousarray(np.asarray(inputs["x"], dtype=np.float32))
    gate_w = np.ascontiguousarray(np.asarray(inputs["gate_w"], dtype=np.float32))
    w1 = np.ascontiguousarray(np.asarray(inputs["w1"], dtype=np.float32))
    w2 = np.ascontiguousarray(np.asarray(inputs["w2"], dtype=np.float32))
    w3 = np.ascontiguousarray(np.asarray(inputs["w3"], dtype=np.float32))
    xf = x.reshape(T, H)

    # capacity safety check (host-side routing estimate; K has margin over
    # the boundary-rounding uncertainty of this estimate)
    logits = xf @ gate_w.T
    m2 = np.sort(logits, axis=1)[:, -2:-1]
    mask = logits >= m2
    pp = mask.reshape(R, RL, E).sum(axis=1)
    if pp.max() > K:
        raise RuntimeError(
            f"per-row expert token count {pp.max()} exceeds compiled "
            f"capacity K={K}; rebuild kernel.py with a larger K"
        )

    if "nc" not in _cache:
        _cache["nc"] = _build_nc()
    nc = _cache["nc"]

    shared = _prep_shared(xf, gate_w, w1, w2, w3)
    in_maps = [_prep_core_inputs(shared, xf, e) for e in range(NCORES)]
    res = run_bass_kernel_spmd(nc, in_maps, core_ids=list(range(NCORES)), trace=trace)

    out = np.zeros((T + 1, H), dtype=np.float32)
    for e in range(NCORES):
        idx = res.results[e]["idx"]
        y = res.results[e]["y"]
        out[idx] += y
    return out[:T].reshape(x.shape), res


def kernel(**inputs) -> np.ndarray:
    out, _ = _run(inputs, trace=False)
    return out
